# revision 2
# baseline (speedup 1.0000x reference)
"""GQA attention forward, sharded head-parallel across 8 Trainium2 NeuronCores.

Full inputs in, full output out. Each core i handles query heads 4i..4i+3 and
KV head i (NH=32, NKV=8, GROUP=4, HD=64):
  - Wq columns 256i:256(i+1), Wk/Wv columns 64i:64(i+1), Wo rows 256i:256(i+1)

Host->device traffic is minimized with on-device collectives:
  - host ships each core only its 256-row feature slice of xT (4MB); cores
    AllGather to the full [2048, 4096] xT in DRAM.
  - each core computes a full-shape partial of out @ Wo into DRAM; a
    ReduceScatter(add) leaves core i with the fully-summed token slice
    512i:512(i+1), to which it adds bo. Host just concatenates slices.

Device pipeline per core (all matmuls fp32r, N=512):
  1. projections: QT [256,4096], KT (duplicated to both partition halves)
     [128,4096], VT [64,4096] -> PE-transposed to token-major V_ones [128,65]
     tiles (ones column for the softmax denominator).
  2. per (batch, head, 512-query-chunk): scoresT [k,q] psum tiles -> exp on ACT
     -> AV accumulation (lhsT=V_ones) giving [attn^T | Z] in psum -> reciprocal
     + broadcast + multiply -> attnT [256,4096].
  3. out partial = attnT.T @ Wo per 128-token tile, DMA to DRAM partial;
     ReduceScatter; + bo; DMA token slice to out.
"""
import sys
import numpy as np

sys.path.insert(0, "/opt/trn_rl_repo")

import concourse.bass as bass
import concourse.tile as tile
from concourse import bacc, mybir
from concourse import bass_utils
from concourse.masks import make_identity

f32 = mybir.dt.float32
f32r = mybir.dt.float32r
bf16 = mybir.dt.bfloat16
AF = mybir.ActivationFunctionType

try:
    import ml_dtypes
    _BF16 = ml_dtypes.bfloat16
except ImportError:  # pragma: no cover
    _BF16 = None


def _to_bf16(a):
    """Fast float32 -> bfloat16 with round-to-nearest-even."""
    b = np.ascontiguousarray(a, np.float32).view(np.uint32)
    r = ((b + (0x7FFF + ((b >> 16) & 1))) >> 16).astype(np.uint16)
    return r.view(_BF16).reshape(a.shape)


def _from_bf16(a):
    u = np.asarray(a).view(np.uint16).astype(np.uint32) << 16
    return u.view(np.float32)

B, S, D = 2, 2048, 2048
NH, NKV, HD = 32, 8, 64
NCORES = 8
HLOC = NH // NCORES           # 4 query heads per core
QF = HLOC * HD                # 256 local q features
N = B * S                     # 4096 tokens
KC = D // 128                 # 16 contraction chunks
NQC = N // 512                # 8 global 512-token chunks
TOK = N // NCORES             # 512 output tokens per core
SCALE = 1.0 / np.sqrt(HD)

_CACHE = {}


def _build():
    nc = bacc.Bacc("TRN2", target_bir_lowering=False, debug=False,
                   num_devices=NCORES)
    xTs_d = nc.dram_tensor("xTs", [QF, N], bf16, kind="ExternalInput").ap()
    wq_d = nc.dram_tensor("Wq", [D, QF], bf16, kind="ExternalInput").ap()
    wk_d = nc.dram_tensor("Wk", [D, HD], bf16, kind="ExternalInput").ap()
    wv_d = nc.dram_tensor("Wv", [D, HD], bf16, kind="ExternalInput").ap()
    wo_d = nc.dram_tensor("Wo", [QF, D], bf16, kind="ExternalInput").ap()
    bq_d = nc.dram_tensor("bq", [1, QF], f32, kind="ExternalInput").ap()
    bk_d = nc.dram_tensor("bk", [1, HD], f32, kind="ExternalInput").ap()
    bv_d = nc.dram_tensor("bv", [1, HD], f32, kind="ExternalInput").ap()
    bo_d = nc.dram_tensor("bo", [1, D], bf16, kind="ExternalInput").ap()
    out_d = nc.dram_tensor("out", [TOK, D], bf16, kind="ExternalOutput").ap()

    RG = [list(range(NCORES))]

    with tile.TileContext(nc) as tc:
        with tc.tile_pool(name="dram", bufs=1, space="DRAM") as dram, \
             tc.tile_pool(name="wpool", bufs=1) as wpool, \
             tc.tile_pool(name="xpool", bufs=4) as xpool, \
             tc.tile_pool(name="big", bufs=1) as big, \
             tc.tile_pool(name="epool", bufs=4) as epool, \
             tc.tile_pool(name="npool", bufs=2) as npool, \
             tc.tile_pool(name="outp", bufs=2) as outp, \
             tc.tile_pool(name="ps_proj", bufs=4, space="PSUM") as ps_proj, \
             tc.tile_pool(name="ps_s", bufs=2, space="PSUM") as ps_s, \
             tc.tile_pool(name="ps_av", bufs=1, space="PSUM") as ps_av, \
             tc.tile_pool(name="ps_o", bufs=1, space="PSUM") as ps_o:

            # ---- phase 0: AllGather x feature slices -> full xT in DRAM ----
            ag_in = dram.tile([QF, N], bf16, name="ag_in")
            xTfull = dram.tile([D, N], bf16, addr_space="Shared", name="xTfull")
            nc.sync.dma_start(ag_in[:], xTs_d[:])
            nc.gpsimd.collective_compute(
                "AllGather", mybir.AluOpType.bypass, replica_groups=RG,
                ins=[ag_in[:]], outs=[xTfull[:]])

            partial = dram.tile([N, D], bf16, name="partial")
            rs_out = dram.tile([TOK, D], bf16, name="rs_out")

            # ---- static tiles -------------------------------------------------
            wq = [wpool.tile([128, QF], bf16, tag=f"wq{k}", name=f"wq{k}") for k in range(KC)]
            wk = [wpool.tile([128, HD], bf16, tag=f"wk{k}", name=f"wk{k}") for k in range(KC)]
            wv = [wpool.tile([128, HD], bf16, tag=f"wv{k}", name=f"wv{k}") for k in range(KC)]
            for k in range(KC):
                nc.sync.dma_start(wq[k][:], wq_d[k * 128:(k + 1) * 128, :])
                nc.sync.dma_start(wk[k][:], wk_d[k * 128:(k + 1) * 128, :])
                nc.sync.dma_start(wv[k][:], wv_d[k * 128:(k + 1) * 128, :])
            wo = [wpool.tile([128, D], bf16, tag=f"wo{m}", name=f"wo{m}") for m in range(2)]
            for m in range(2):
                nc.sync.dma_start(wo[m][:], wo_d[m * 128:(m + 1) * 128, :])
            bq = wpool.tile([1, QF], f32, tag="bq")
            bk = wpool.tile([1, HD], f32, tag="bk")
            bv = wpool.tile([1, HD], f32, tag="bv")
            nc.sync.dma_start(bq[:].bitcast(f32r), bq_d[:].bitcast(f32r))
            nc.sync.dma_start(bk[:].bitcast(f32r), bk_d[:].bitcast(f32r))
            nc.sync.dma_start(bv[:].bitcast(f32r), bv_d[:].bitcast(f32r))
            # bo arrives pre-scaled by 1/NCORES; added into every core's
            # partial so the ReduceScatter sum reconstructs bo exactly.
            bo1 = wpool.tile([1, D], bf16, tag="bo1")
            nc.sync.dma_start(bo1[:], bo_d[:])
            ones_raw = wpool.tile([128, 512], f32, tag="ones_raw")
            nc.gpsimd.memset(ones_raw[:], 1.0)
            ones = wpool.tile([1, 512], f32, tag="ones")
            nc.vector.tensor_copy(ones[:].bitcast(f32r), ones_raw[0:1, :])
            ones_h = wpool.tile([1, 128], bf16, tag="ones_h")
            nc.vector.tensor_copy(ones_h[:], ones_raw[0:1, 0:128])
            ident = wpool.tile([64, 64], f32, tag="ident")
            make_identity(nc, ident[:])

            qt = [big.tile([128, N], f32, tag=f"qt{m}", name=f"qt{m}") for m in range(2)]
            ktd = big.tile([128, N], f32, tag="ktd")
            vt = big.tile([64, N], f32, tag="vt")
            vones = [big.tile([128, 16 * 65], f32, tag=f"vo{b}", name=f"vo{b}") for b in range(B)]
            for b in range(B):
                vo3 = vones[b].rearrange("p (t c) -> p t c", c=65)
                nc.vector.tensor_copy(vo3[:, :, 64:65].bitcast(f32r),
                                      ones_raw[:, 0:16].unsqueeze(2))
            attnT = [big.tile([128, N], bf16, tag=f"at{m}", name=f"at{m}") for m in range(2)]

            # ---- phase 1: projections ----------------------------------------
            for qc in range(NQC):
                cs = slice(qc * 512, (qc + 1) * 512)
                psq = [ps_proj.tile([128, 512], f32, tag="pp", name="psq") for _ in range(2)]
                psk = ps_proj.tile([64, 512], f32, tag="pp")
                psv = ps_proj.tile([64, 512], f32, tag="pp")
                for m in range(2):
                    nc.tensor.matmul(psq[m][:], bq[0:1, m * 128:(m + 1) * 128].bitcast(f32r),
                                     ones[:].bitcast(f32r), start=True, stop=False)
                nc.tensor.matmul(psk[:], bk[:].bitcast(f32r), ones[:].bitcast(f32r),
                                 start=True, stop=False)
                nc.tensor.matmul(psv[:], bv[:].bitcast(f32r), ones[:].bitcast(f32r),
                                 start=True, stop=False)
                for k in range(KC):
                    xt = xpool.tile([128, 512], bf16, tag="xt")
                    nc.sync.dma_start(xt[:], xTfull[k * 128:(k + 1) * 128, cs])
                    last = k == KC - 1
                    for m in range(2):
                        nc.tensor.matmul(psq[m][:],
                                         wq[k][:, m * 128:(m + 1) * 128],
                                         xt[:], start=False, stop=last)
                    nc.tensor.matmul(psk[:], wk[k][:], xt[:], start=False, stop=last)
                    nc.tensor.matmul(psv[:], wv[k][:], xt[:], start=False, stop=last)
                for m in range(2):
                    nc.scalar.copy(qt[m][:, cs].bitcast(f32r), psq[m][:])
                nc.scalar.copy(ktd[0:64, cs].bitcast(f32r), psk[:])
                nc.sync.dma_start(ktd[64:128, cs].bitcast(f32r), ktd[0:64, cs].bitcast(f32r))
                nc.scalar.copy(vt[:, cs], psv[:])

            # ---- phase 1b: V transpose to token-major ------------------------
            for b in range(B):
                for kt in range(16):
                    pst = ps_proj.tile([128, 64], f32, tag="pp")
                    src = vt[:, b * S + kt * 128: b * S + (kt + 1) * 128]
                    nc.tensor.transpose(pst[:], src, ident[:])
                    nc.vector.tensor_copy(vones[b][:, kt * 65: kt * 65 + 64].bitcast(f32r), pst[:])

            # ---- phase 2: attention + output projection ----------------------
            for b in range(B):
                for qcl in range(4):
                    qcg = b * 4 + qcl
                    cs = slice(qcg * 512, (qcg + 1) * 512)
                    for h in range(HLOC):
                        m, r = h // 2, h % 2
                        base = r * 64
                        psav = ps_av.tile([65, 512], f32, tag="av")
                        for kt in range(16):
                            pss = ps_s.tile([128, 512], f32, tag="s")
                            nc.tensor.matmul(
                                pss[:],
                                ktd[base:base + 64,
                                    b * S + kt * 128: b * S + (kt + 1) * 128].bitcast(f32r),
                                qt[m][base:base + 64, cs].bitcast(f32r),
                                start=True, stop=True)
                            es = epool.tile([128, 512], f32, tag="es")
                            nc.scalar.activation(es[:].bitcast(f32r), pss[:], AF.Exp, scale=float(SCALE))
                            nc.tensor.matmul(
                                psav[:],
                                vones[b][:, kt * 65: kt * 65 + 65].bitcast(f32r),
                                es[:].bitcast(f32r),
                                start=(kt == 0), stop=(kt == 15))
                        rec65 = npool.tile([65, 512], f32, tag="rec")
                        nc.vector.reciprocal(rec65[:], psav[:])
                        rz0 = npool.tile([1, 512], f32, tag="z0")
                        nc.sync.dma_start(rz0[:], rec65[64:65, :])
                        rzb = npool.tile([64, 512], f32, tag="rzb")
                        nc.gpsimd.partition_broadcast(rzb[:], rz0[:])
                        if r == 0:
                            nc.vector.tensor_mul(attnT[m][0:64, cs],
                                                 psav[0:64, :], rzb[:])
                        else:
                            tmp = npool.tile([64, 512], bf16, tag="tmp")
                            nc.vector.tensor_mul(tmp[:], psav[0:64, :], rzb[:])
                            nc.sync.dma_start(attnT[m][64:128, cs], tmp[:])
                    for t in range(4):
                        tok = qcg * 512 + t * 128
                        osb = outp.tile([128, D], bf16, tag="osb")
                        for oc in range(4):
                            pso = ps_o.tile([128, 512], f32, tag="o")
                            nc.tensor.matmul(pso[:], ones_h[:],
                                             bo1[:, oc * 512:(oc + 1) * 512],
                                             start=True, stop=False)
                            for m in range(2):
                                nc.tensor.matmul(
                                    pso[:],
                                    attnT[m][:, tok:tok + 128],
                                    wo[m][:, oc * 512:(oc + 1) * 512],
                                    start=False, stop=(m == 1))
                            nc.vector.tensor_copy(osb[:, oc * 512:(oc + 1) * 512], pso[:])
                        nc.sync.dma_start(partial[tok:tok + 128, :], osb[:])

            # ---- phase 3: ReduceScatter + output -----------------------------
            nc.gpsimd.collective_compute(
                "ReduceScatter", mybir.AluOpType.add, replica_groups=RG,
                ins=[partial[:]], outs=[rs_out[:]])
            nc.sync.dma_start(out_d[:], rs_out[:])

    nc.compile()
    return nc


def kernel(x, Wq, bq, Wk, bk, Wv, bv, Wo, bo, _trace=False):
    x = np.asarray(x, np.float32)
    xTh = _to_bf16(x.reshape(N, D).T)
    Wqh = _to_bf16(Wq)
    Wkh = _to_bf16(Wk)
    Wvh = _to_bf16(Wv)
    Woh = _to_bf16(Wo)
    bo2 = _to_bf16(np.asarray(bo, np.float32).reshape(1, D) / NCORES)
    in_maps = []
    for i in range(NCORES):
        in_maps.append({
            "xTs": xTh[i * QF:(i + 1) * QF],
            "Wq": Wqh[:, i * QF:(i + 1) * QF],
            "Wk": Wkh[:, i * HD:(i + 1) * HD],
            "Wv": Wvh[:, i * HD:(i + 1) * HD],
            "Wo": Woh[i * QF:(i + 1) * QF, :],
            "bq": np.ascontiguousarray(bq[i * QF:(i + 1) * QF].reshape(1, QF), np.float32),
            "bk": np.ascontiguousarray(bk[i * HD:(i + 1) * HD].reshape(1, HD), np.float32),
            "bv": np.ascontiguousarray(bv[i * HD:(i + 1) * HD].reshape(1, HD), np.float32),
            "bo": bo2,
        })
    if "nc" not in _CACHE:
        _CACHE["nc"] = _build()
    nc = _CACHE["nc"]
    res = bass_utils.run_bass_kernel_spmd(nc, in_maps, core_ids=list(range(NCORES)),
                                          trace=_trace)
    _CACHE["last_result"] = res
    out = np.concatenate([res.results[i]["out"] for i in range(NCORES)], axis=0)
    return _from_bf16(out).reshape(B, S, D)


if __name__ == "__main__":
    rng = np.random.default_rng(1)
    inputs = {
        "x": rng.standard_normal((B, S, D)).astype(np.float32),
        "Wq": (rng.standard_normal((D, D)) * 0.01).astype(np.float32),
        "bq": (rng.standard_normal((D,)) * 0.01).astype(np.float32),
        "Wk": (rng.standard_normal((D, NKV * HD)) * 0.01).astype(np.float32),
        "bk": (rng.standard_normal((NKV * HD,)) * 0.01).astype(np.float32),
        "Wv": (rng.standard_normal((D, NKV * HD)) * 0.01).astype(np.float32),
        "bv": (rng.standard_normal((NKV * HD,)) * 0.01).astype(np.float32),
        "Wo": (rng.standard_normal((D, D)) * 0.01).astype(np.float32),
        "bo": (rng.standard_normal((D,)) * 0.01).astype(np.float32),
    }
    out = kernel(**inputs)
    print("kernel ran, out shape", out.shape)
